# revision 4
# baseline (speedup 1.0000x reference)
"""GNN message-passing aggregator (bi-interaction) on 8 TRN2 NeuronCores.

Strategy: destination-node sharding. Core c owns nodes [c*6250, (c+1)*6250).
Each core processes exactly the edges whose dst lands in its shard, so each
core's segment-sum is complete (no cross-core reduction needed) and each core
emits its own contiguous slice of the output.

Per-core device pipeline (single shared NEFF, SPMD over 8 cores):
  1. dma_gather src embeddings (256B bf16 rows) from HBM, 4 SWDGE queues
     (greedy-balanced by descriptor count). The gather is Q7
     descriptor-generation-bound (~6ns/row, independent of row bytes), so
     the schedule minimizes gathered slots: edges are packed into 128-slot
     quanta per (128-node dst window, src-range class) bucket, padded to
     the max count over the 8 cores. int16 gather indices can't span 50000
     rows, so edges are classed by src < 32768 (lo) / >= 32768 (hi) and
     gathered from two table bases. A deep xg tile pool lets desc-gen run
     ahead across region boundaries.
  2. Per chunk (up to 8 quanta), the [128 x ncols x 128] bf16 selection
     tensor S (S[slot, q, dst_off] = att) is built with two chunk-wide DVE
     tensor_tensor ops against broadcast (stride-0) APs:
       S = is_equal(iota[j], c[p,q]) ; S *= att[p,q]
     (NOT per-quantum tensor_scalar with SBUF-pointer scalars, which costs
     ~1.5us/op on HW fetching 128 per-partition scalars.)
     Each 128-slot quantum is one PE matmul X_q.T @ S_q (bf16 in, fp32
     PSUM) accumulating into its window's 128-column slice of a PSUM bank;
     one bank holds a 512-node region. Banks are pre-zeroed and all
     matmuls accumulate, because start=True clears has_written for the
     whole bank, which would break interleaved per-window accumulation.
  3. Interleaved per region, phase C reads the PSUM bank directly:
     H1 = x+N_h, H2 = x*N_h; out = lrelu(W1@H1+b1) + lrelu(W2@H2+b2) with
     fp32 matmuls (lhsT = W.T in SBUF). x is uploaded pre-transposed
     (feature-major) and the output is written feature-major and
     transposed back on the host, so no PE transposes are needed anywhere.
     x and the phase-3 math stay fp32; only the per-edge message path is
     bf16 (rel err ~2e-3 vs 2e-2 tolerance).
"""

import os
import numpy as np

import concourse.bacc as bacc
import concourse.mybir as mybir
from concourse.tile import TileContext
from concourse.bass_utils import run_bass_kernel_spmd

N_NODES = 50000
N_EDGES = 600000
D = 128
N_CORES = 8
NPC = N_NODES // N_CORES          # 6250 nodes per core
NPC_PAD = 6272                    # 49 * 128
W = 128                           # destination window width
NWIN = NPC_PAD // W               # 49 windows
REGION = 512                      # nodes per PSUM bank
NREG = (NPC_PAD + REGION - 1) // REGION   # 13 (last region holds 128 cols)
SPLIT = 32768                     # int16 gather index limit
Q = 128                           # slot quantum (= one full PE tile)
LEAKY = 0.01
F32 = mybir.dt.float32
BF16 = mybir.dt.bfloat16
I16 = mybir.dt.int16
NP_BF16 = mybir.dt.np(BF16)


def _build_schedule(src_idx, dst_idx, att):
    """Host-side shared schedule (quantum-granular) + per-core data arrays."""
    cap = 128 * int(os.environ.get("GNN_MAXGT", "8"))
    gq = int(os.environ.get("GNN_GQUEUES", "4"))
    src_idx = np.asarray(src_idx, dtype=np.int64)
    dst_idx = np.asarray(dst_idx, dtype=np.int64)
    att = np.asarray(att, dtype=np.float32)

    core_of = dst_idx // NPC
    counts = np.zeros((N_CORES, NWIN, 2), dtype=np.int64)
    per_core = []
    for c in range(N_CORES):
        m = core_of == c
        es, ed, ea = src_idx[m], dst_idx[m] - c * NPC, att[m]
        wi = ed // W
        cl = (es >= SPLIT).astype(np.int64)
        np.add.at(counts, (c, wi, cl), 1)
        per_core.append((es, ed, ea, wi, cl))

    # bucket (w, cls) slot count: shared over cores, quantum granular
    bslots = -(-counts.max(axis=0) // Q) * Q          # [NWIN, 2]

    # Global slot sequence: per region, lo buckets (w asc), then hi buckets.
    # Chunk = one dma_gather: same-class slot run <= cap.
    chunks = []
    bucket_slot_base = np.full((NWIN, 2), -1, dtype=np.int64)
    slot_pos = 0
    col_pos = 0
    last_q_of_w = {}
    qload = [0] * gq                  # greedy queue balancing by desc count

    wpr = REGION // W                 # windows per region
    for r in range(NREG):
        ws = [w_ for w_ in range(r * wpr, min((r + 1) * wpr, NWIN))]
        for cls in (0, 1):
            todo = [[wdx, int(bslots[wdx, cls])] for wdx in ws
                    if bslots[wdx, cls] > 0]
            i = 0
            while i < len(todo):
                n = 0
                quanta = []          # (col_local, row_grp, wdx)
                while i < len(todo) and n < cap:
                    wdx, bn = todo[i]
                    take = min(bn, cap - n)
                    if bucket_slot_base[wdx, cls] < 0:
                        bucket_slot_base[wdx, cls] = slot_pos + n
                    for q0 in range(0, take, Q):
                        j = n + q0
                        quanta.append((j // 128, (j % 128) // Q, wdx))
                    if take == bn:
                        i += 1
                    else:
                        todo[i][1] = bn - take
                    n += take
                ncols = -(-n // 128)
                ci = len(chunks)
                for qi, (qc, qa, qw) in enumerate(quanta):
                    last_q_of_w[qw] = (ci, qi)
                qn = min(range(gq), key=lambda k: qload[k])
                qload[qn] += n
                chunks.append(dict(cls=cls, slot_base=slot_pos, n=n,
                                   col_base=col_pos, ncols=ncols,
                                   region=r, quanta=quanta, queue=qn))
                # reserve whole columns; tail slots [n, ncols*128) get
                # idx=-1 (trailing negatives -> descriptors skipped)
                slot_pos += ncols * 128
                col_pos += ncols

    total_slots = slot_pos
    total_cols = col_pos

    # slot -> (global col, partition) and slot -> window maps
    slot_col = np.zeros(total_slots, dtype=np.int64)
    slot_part = np.zeros(total_slots, dtype=np.int64)
    slot_win = np.zeros(total_slots, dtype=np.int64)
    for ch in chunks:
        nres = ch["ncols"] * 128
        j = np.arange(nres)
        sl = slice(ch["slot_base"], ch["slot_base"] + nres)
        slot_col[sl] = ch["col_base"] + j // 128
        slot_part[sl] = j % 128
        for (qc, qa, qw) in ch["quanta"]:
            s0 = ch["slot_base"] + qc * 128 + qa * Q
            slot_win[s0:s0 + Q] = qw

    # per-core slot data
    idx_arrs, att_arrs, c_arrs = [], [], []
    for c in range(N_CORES):
        es, ed, ea, wi, cl = per_core[c]
        bucket = wi * 2 + cl
        order = np.argsort(bucket, kind="stable")
        b_sorted = bucket[order]
        start_marks = np.r_[True, b_sorted[1:] != b_sorted[:-1]]
        grp_starts = np.flatnonzero(start_marks)
        rank = np.arange(len(b_sorted)) - np.repeat(
            grp_starts, np.diff(np.r_[grp_starts, len(b_sorted)]))
        slots = bucket_slot_base[wi[order], cl[order]] + rank

        idx_full = np.zeros(total_slots, dtype=np.int16)
        for ch in chunks:
            idx_full[ch["slot_base"] + ch["n"]:
                     ch["slot_base"] + ch["ncols"] * 128] = -1
        att_full = np.zeros(total_slots, dtype=np.float32)
        c_full = np.zeros(total_slots, dtype=np.float32)
        idx_full[slots] = (es[order] - cl[order] * SPLIT).astype(np.int16)
        att_full[slots] = ea[order]
        c_full[slots] = (ed[order] - slot_win[slots] * W).astype(np.float32)

        # idx wrapped: chunk-local slot j -> [j % 16, j // 16]; every chunk's
        # slot_base is a multiple of 16, so global wrapping == chunk-local.
        idx_w = np.tile(idx_full.reshape(total_slots // 16, 16).T, (8, 1))
        att_a = np.zeros((128, total_cols), dtype=np.float32)
        c_a = np.zeros((128, total_cols), dtype=np.float32)
        att_a[slot_part, slot_col] = att_full
        c_a[slot_part, slot_col] = c_full
        idx_arrs.append(np.ascontiguousarray(idx_w))
        att_arrs.append(np.ascontiguousarray(att_a))
        c_arrs.append(np.ascontiguousarray(c_a.astype(NP_BF16)))

    sched = {
        "total_slots": total_slots,
        "total_cols": total_cols,
        "chunks": chunks,
        "last_q_of_w": last_q_of_w,
    }
    return sched, idx_arrs, att_arrs, c_arrs


def _trace(sched):
    total_slots = sched["total_slots"]
    total_cols = sched["total_cols"]
    chunks = sched["chunks"]
    last_q_of_w = sched["last_q_of_w"]
    maxgt = int(os.environ.get("GNN_MAXGT", "8"))

    gq = int(os.environ.get("GNN_GQUEUES", "4"))
    scratch = int(os.environ.get("GNN_SCRATCH", "32768"))
    nc = bacc.Bacc("TRN2", debug=False, num_swdge_queues=gq,
                   dynamic_dma_scratch_size=scratch)
    embed_d = nc.dram_tensor("embed_d", [N_NODES, D], BF16,
                             kind="ExternalInput")
    x_d = nc.dram_tensor("x_d", [D, NPC_PAD], F32, kind="ExternalInput")
    idx_d = nc.dram_tensor("idx_d", [128, total_slots // 16], I16,
                           kind="ExternalInput")
    att_d = nc.dram_tensor("att_d", [128, total_cols], F32,
                           kind="ExternalInput")
    c_d = nc.dram_tensor("c_d", [128, total_cols], BF16,
                         kind="ExternalInput")
    w1t_d = nc.dram_tensor("w1t_d", [D, D], F32, kind="ExternalInput")
    w2t_d = nc.dram_tensor("w2t_d", [D, D], F32, kind="ExternalInput")
    b1_d = nc.dram_tensor("b1_d", [D, 1], F32, kind="ExternalInput")
    b2_d = nc.dram_tensor("b2_d", [D, 1], F32, kind="ExternalInput")
    out_d = nc.dram_tensor("out_d", [D, NPC], F32, kind="ExternalOutput")

    iota_np = np.tile(np.arange(W, dtype=np.float32), (128, 1)).astype(NP_BF16)

    with TileContext(nc) as tc:
        with (
            tc.tile_pool(name="aux", bufs=1) as aux,
            tc.tile_pool(name="xgp", bufs=int(os.environ.get("GNN_XGBUFS", "24"))) as xgp,
            tc.tile_pool(name="sp", bufs=int(os.environ.get("GNN_SPBUFS", "12"))) as sp,
            tc.tile_pool(name="hp", bufs=3) as hp,
            tc.tile_pool(name="ps", bufs=1, space="PSUM") as ps,
        ):
            idx_t = aux.tile([128, total_slots // 16], I16)
            nc.sync.dma_start(idx_t[:], idx_d[:])
            att_t = aux.tile([128, total_cols], F32)
            nc.sync.dma_start(att_t[:], att_d[:])
            c_t = aux.tile([128, total_cols], BF16)
            nc.sync.dma_start(c_t[:], c_d[:])
            iota_t = aux.tile([128, W], BF16)
            nc.sync.dma_start(iota_t[:], nc.inline_tensor(iota_np, "iota_i")[:])
            w1t_t = aux.tile([D, D], F32)
            nc.sync.dma_start(w1t_t[:], w1t_d[:])
            w2t_t = aux.tile([D, D], F32)
            nc.sync.dma_start(w2t_t[:], w2t_d[:])
            b1_t = aux.tile([D, 1], F32)
            nc.sync.dma_start(b1_t[:], b1_d[:])
            b2_t = aux.tile([D, 1], F32)
            nc.sync.dma_start(b2_t[:], b2_d[:])

            x_fm = aux.tile([128, NPC_PAD], F32)     # x feature-major

            phases = os.environ.get("GNN_PHASES", "ABC")

            def emit_region(r, bank):
                for ci, ch in enumerate(chunks):
                    if ch["region"] != r:
                        continue
                    n, ncols, cb = ch["n"], ch["ncols"], ch["col_base"]
                    xg = xgp.tile([128, maxgt, D], BF16, tag="xg")
                    base_ap = (embed_d[:] if ch["cls"] == 0
                               else embed_d[SPLIT:])
                    nc.gpsimd.dma_gather(
                        xg[:, :ncols, :], base_ap,
                        idx_t[:, ch["slot_base"] // 16:
                              ch["slot_base"] // 16 + ncols * 8],
                        ncols * 128, n, D,
                        elem_step=D,
                        queue_num=ch["queue"])
                    # chunk-wide S build: S[p, q, j] = (iota[j]==c[p,q])*att
                    s_t = sp.tile([128, maxgt, W], BF16, tag="s")
                    iota_b = iota_t[:].unsqueeze(1).to_broadcast(
                        [128, ncols, W])
                    c_b = c_t[:, cb:cb + ncols].unsqueeze(2).to_broadcast(
                        [128, ncols, W])
                    att_b = att_t[:, cb:cb + ncols].unsqueeze(2).to_broadcast(
                        [128, ncols, W])
                    nc.vector.tensor_tensor(
                        s_t[:, :ncols, :], iota_b, c_b,
                        op=mybir.AluOpType.is_equal)
                    if os.environ.get("GNN_SMUL", "scalar") == "vector":
                        nc.vector.tensor_tensor(
                            s_t[:, :ncols, :], s_t[:, :ncols, :], att_b,
                            op=mybir.AluOpType.mult)
                    else:
                        # per-quantum att multiply on the Activation engine:
                        # per-partition scale from SBUF is the fast path there
                        # (unlike DVE tensor_scalar ptr fetches).
                        for qc2 in range(ncols):
                            nc.scalar.activation(
                                s_t[:, qc2, :], s_t[:, qc2, :],
                                mybir.ActivationFunctionType.Copy,
                                scale=att_t[:, cb + qc2:cb + qc2 + 1])
                    for qi, (qc, qa, qw) in enumerate(ch["quanta"]):
                        wl = qw % (REGION // W)
                        nc.tensor.matmul(
                            bank[:, wl * W:(wl + 1) * W],
                            lhsT=xg[:, qc, :],
                            rhs=s_t[:, qc, :],
                            start=False,
                            stop=(last_q_of_w[qw] == (ci, qi)),
                            skip_group_check=True)

            def emit_c_chunk(k, bank):
                g0 = k * REGION
                n = min(REGION, NPC_PAD - g0)
                sl = slice(g0, g0 + n)
                h1 = hp.tile([128, REGION], F32, tag="h1")
                nc.vector.tensor_add(h1[:, :n], x_fm[:, sl], bank[:, :n])
                h2 = hp.tile([128, REGION], F32, tag="h2")
                nc.vector.tensor_mul(h2[:, :n], x_fm[:, sl], bank[:, :n])
                pf1 = ps.tile([128, REGION], F32, tag="fin", bufs=2)
                nc.tensor.matmul(pf1[:, :n], lhsT=w1t_t[:], rhs=h1[:, :n],
                                 start=True, stop=True)
                o1 = hp.tile([128, REGION], F32, tag="o1")
                nc.scalar.activation(
                    o1[:, :n], pf1[:, :n], mybir.ActivationFunctionType.Lrelu,
                    bias=b1_t[:, 0:1], scale=1.0, alpha=LEAKY)
                pf2 = ps.tile([128, REGION], F32, tag="fin", bufs=2)
                nc.tensor.matmul(pf2[:, :n], lhsT=w2t_t[:], rhs=h2[:, :n],
                                 start=True, stop=True)
                o2 = hp.tile([128, REGION], F32, tag="o2")
                nc.scalar.activation(
                    o2[:, :n], pf2[:, :n], mybir.ActivationFunctionType.Lrelu,
                    bias=b2_t[:, 0:1], scale=1.0, alpha=LEAKY)
                osum = hp.tile([128, REGION], F32, tag="os")
                nc.vector.tensor_add(osum[:, :n], o1[:, :n], o2[:, :n])
                rows = min(REGION, NPC - g0)
                if rows > 0:
                    nc.sync.dma_start(out_d[:, g0:g0 + rows],
                                      osum[:, :rows])

            def body():
                if "A" in phases:
                    nc.sync.dma_start(x_fm[:], x_d[:])
                for r in range(NREG):
                    if "B" in phases:
                        bank = ps.tile([128, REGION], F32, tag="bank", bufs=4)
                        nc.scalar.memzero(bank[:])
                        emit_region(r, bank)
                    if "C" in phases and "B" in phases:
                        emit_c_chunk(r, bank)

            body()

    nc.compile()
    return nc


def kernel(entity_embed, att, src_idx, dst_idx, W1, b1, W2, b2):
    entity_embed = np.ascontiguousarray(entity_embed, dtype=np.float32)
    att = np.ascontiguousarray(att, dtype=np.float32)
    src_idx_np = np.ascontiguousarray(src_idx, dtype=np.int32)
    dst_idx_np = np.ascontiguousarray(dst_idx, dtype=np.int32)

    sched, idx_arrs, att_arrs, c_arrs = _build_schedule(
        src_idx_np, dst_idx_np, att)
    nc = _trace(sched)

    w1t = np.ascontiguousarray(np.asarray(W1, np.float32).T)
    w2t = np.ascontiguousarray(np.asarray(W2, np.float32).T)
    b1c = np.ascontiguousarray(np.asarray(b1, np.float32).reshape(D, 1))
    b2c = np.ascontiguousarray(np.asarray(b2, np.float32).reshape(D, 1))

    embed_bf16 = np.ascontiguousarray(entity_embed.astype(NP_BF16))
    in_maps = []
    for c in range(N_CORES):
        x_slice = np.zeros((NPC_PAD, D), np.float32)
        x_slice[:NPC] = entity_embed[c * NPC:(c + 1) * NPC]
        in_maps.append({
            "embed_d": embed_bf16,
            "x_d": np.ascontiguousarray(x_slice.T),
            "idx_d": idx_arrs[c],
            "att_d": att_arrs[c],
            "c_d": c_arrs[c],
            "w1t_d": w1t,
            "w2t_d": w2t,
            "b1_d": b1c,
            "b2_d": b2c,
        })

    trace = os.environ.get("GNN_KERNEL_TRACE", "0") == "1"
    res = run_bass_kernel_spmd(
        nc, in_maps, core_ids=list(range(N_CORES)), trace=trace)
    if trace and res.exec_time_ns is not None:
        print(f"HW exec time: {res.exec_time_ns} ns")

    return np.concatenate(
        [np.ascontiguousarray(res.results[c]["out_d"].T)
         for c in range(N_CORES)], axis=0)


# revision 5
# speedup vs baseline: 1.2372x; 1.2372x over previous
"""GNN message-passing aggregator (bi-interaction) on 8 TRN2 NeuronCores.

Strategy: destination-node sharding. Core c owns nodes [c*6250, (c+1)*6250).
Each core processes exactly the edges whose dst lands in its shard, so each
core's segment-sum is complete (no cross-core reduction needed) and each core
emits its own contiguous slice of the output.

Per-core device pipeline (single shared NEFF, SPMD over 8 cores):
  1. dma_gather src embeddings (256B bf16 rows) from HBM, 4 SWDGE queues
     (greedy-balanced by descriptor count). The gather is Q7
     descriptor-generation-bound (~6ns/row, independent of row bytes), so
     the schedule minimizes gathered slots: edges are packed into 128-slot
     quanta per (128-node dst window, src-range class) bucket, padded to
     the max count over the 8 cores. int16 gather indices can't span 50000
     rows, so edges are classed by src < 32768 (lo) / >= 32768 (hi) and
     gathered from two table bases. A deep xg tile pool lets desc-gen run
     ahead across region boundaries.
  2. Per chunk (up to 8 quanta), the [128 x ncols x 128] bf16 selection
     tensor S (S[slot, q, dst_off] = att) is built with two chunk-wide DVE
     tensor_tensor ops against broadcast (stride-0) APs:
       S = is_equal(iota[j], c[p,q]) ; S *= att[p,q]
     (NOT per-quantum tensor_scalar with SBUF-pointer scalars, which costs
     ~1.5us/op on HW fetching 128 per-partition scalars.)
     Each 128-slot quantum is one PE matmul X_q.T @ S_q (bf16 in, fp32
     PSUM) accumulating into its window's 128-column slice of a PSUM bank;
     one bank holds a 512-node region. Banks are pre-zeroed and all
     matmuls accumulate, because start=True clears has_written for the
     whole bank, which would break interleaved per-window accumulation.
  3. Interleaved per region, phase C reads the PSUM bank directly:
     H1 = x+N_h, H2 = x*N_h; out = lrelu(W1@H1+b1) + lrelu(W2@H2+b2) with
     fp32 matmuls (lhsT = W.T in SBUF). x is uploaded pre-transposed
     (feature-major) and the output is written feature-major and
     transposed back on the host, so no PE transposes are needed anywhere.
     x and the phase-3 math stay fp32; only the per-edge message path is
     bf16 (rel err ~2e-3 vs 2e-2 tolerance).
"""

import os
import numpy as np

import concourse.bacc as bacc
import concourse.mybir as mybir
from concourse.tile import TileContext
from concourse.bass_utils import run_bass_kernel_spmd

N_NODES = 50000
N_EDGES = 600000
D = 128
N_CORES = 8
NPC = N_NODES // N_CORES          # 6250 nodes per core
NPC_PAD = 6272                    # 49 * 128
W = 128                           # destination window width
NWIN = NPC_PAD // W               # 49 windows
REGION = 512                      # nodes per PSUM bank
NREG = (NPC_PAD + REGION - 1) // REGION   # 13 (last region holds 128 cols)
SPLIT = 32768                     # int16 gather index limit
Q = 128                           # slot quantum (= one full PE tile)
LEAKY = 0.01
F32 = mybir.dt.float32
BF16 = mybir.dt.bfloat16
I16 = mybir.dt.int16
NP_BF16 = mybir.dt.np(BF16)


def _build_schedule(src_idx, dst_idx, att):
    """Host-side shared schedule (quantum-granular) + per-core data arrays."""
    cap = 128 * int(os.environ.get("GNN_MAXGT", "8"))
    gq = int(os.environ.get("GNN_GQUEUES", "4"))
    src_idx = np.asarray(src_idx, dtype=np.int64)
    dst_idx = np.asarray(dst_idx, dtype=np.int64)
    att = np.asarray(att, dtype=np.float32)

    core_of = dst_idx // NPC
    counts = np.zeros((N_CORES, NWIN, 2), dtype=np.int64)
    per_core = []
    for c in range(N_CORES):
        m = core_of == c
        es, ed, ea = src_idx[m], dst_idx[m] - c * NPC, att[m]
        wi = ed // W
        cl = (es >= SPLIT).astype(np.int64)
        np.add.at(counts, (c, wi, cl), 1)
        per_core.append((es, ed, ea, wi, cl))

    # bucket (w, cls) slot count: shared over cores, quantum granular
    bslots = -(-counts.max(axis=0) // Q) * Q          # [NWIN, 2]

    # Global slot sequence: per region, lo buckets (w asc), then hi buckets.
    # Chunk = one dma_gather: same-class slot run <= cap.
    chunks = []
    bucket_slot_base = np.full((NWIN, 2), -1, dtype=np.int64)
    slot_pos = 0
    col_pos = 0
    last_q_of_w = {}
    qload = [0] * gq                  # greedy queue balancing by desc count

    wpr = REGION // W                 # windows per region
    for r in range(NREG):
        ws = [w_ for w_ in range(r * wpr, min((r + 1) * wpr, NWIN))]
        for cls in (0, 1):
            todo = [[wdx, int(bslots[wdx, cls])] for wdx in ws
                    if bslots[wdx, cls] > 0]
            i = 0
            while i < len(todo):
                n = 0
                quanta = []          # (col_local, row_grp, wdx)
                while i < len(todo) and n < cap:
                    wdx, bn = todo[i]
                    take = min(bn, cap - n)
                    if bucket_slot_base[wdx, cls] < 0:
                        bucket_slot_base[wdx, cls] = slot_pos + n
                    for q0 in range(0, take, Q):
                        j = n + q0
                        quanta.append((j // 128, (j % 128) // Q, wdx))
                    if take == bn:
                        i += 1
                    else:
                        todo[i][1] = bn - take
                    n += take
                ncols = -(-n // 128)
                ci = len(chunks)
                for qi, (qc, qa, qw) in enumerate(quanta):
                    last_q_of_w[qw] = (ci, qi)
                qn = min(range(gq), key=lambda k: qload[k])
                qload[qn] += n
                chunks.append(dict(cls=cls, slot_base=slot_pos, n=n,
                                   col_base=col_pos, ncols=ncols,
                                   region=r, quanta=quanta, queue=qn))
                # reserve whole columns; tail slots [n, ncols*128) get
                # idx=-1 (trailing negatives -> descriptors skipped)
                slot_pos += ncols * 128
                col_pos += ncols

    total_slots = slot_pos
    total_cols = col_pos

    # slot -> (global col, partition) and slot -> window maps
    slot_col = np.zeros(total_slots, dtype=np.int64)
    slot_part = np.zeros(total_slots, dtype=np.int64)
    slot_win = np.zeros(total_slots, dtype=np.int64)
    for ch in chunks:
        nres = ch["ncols"] * 128
        j = np.arange(nres)
        sl = slice(ch["slot_base"], ch["slot_base"] + nres)
        slot_col[sl] = ch["col_base"] + j // 128
        slot_part[sl] = j % 128
        for (qc, qa, qw) in ch["quanta"]:
            s0 = ch["slot_base"] + qc * 128 + qa * Q
            slot_win[s0:s0 + Q] = qw

    # per-core slot data
    idx_arrs, att_arrs, c_arrs = [], [], []
    for c in range(N_CORES):
        es, ed, ea, wi, cl = per_core[c]
        bucket = wi * 2 + cl
        order = np.argsort(bucket, kind="stable")
        b_sorted = bucket[order]
        start_marks = np.r_[True, b_sorted[1:] != b_sorted[:-1]]
        grp_starts = np.flatnonzero(start_marks)
        rank = np.arange(len(b_sorted)) - np.repeat(
            grp_starts, np.diff(np.r_[grp_starts, len(b_sorted)]))
        slots = bucket_slot_base[wi[order], cl[order]] + rank

        idx_full = np.zeros(total_slots, dtype=np.int16)
        for ch in chunks:
            idx_full[ch["slot_base"] + ch["n"]:
                     ch["slot_base"] + ch["ncols"] * 128] = -1
        att_full = np.zeros(total_slots, dtype=np.float32)
        c_full = np.zeros(total_slots, dtype=np.float32)
        idx_full[slots] = (es[order] - cl[order] * SPLIT).astype(np.int16)
        att_full[slots] = ea[order]
        c_full[slots] = (ed[order] - slot_win[slots] * W).astype(np.float32)

        # idx wrapped: chunk-local slot j -> [j % 16, j // 16]; every chunk's
        # slot_base is a multiple of 16, so global wrapping == chunk-local.
        idx_w = np.tile(idx_full.reshape(total_slots // 16, 16).T, (8, 1))
        att_a = np.zeros((128, total_cols), dtype=np.float32)
        c_a = np.zeros((128, total_cols), dtype=np.float32)
        att_a[slot_part, slot_col] = att_full
        c_a[slot_part, slot_col] = c_full
        idx_arrs.append(np.ascontiguousarray(idx_w))
        att_arrs.append(np.ascontiguousarray(att_a))
        c_arrs.append(np.ascontiguousarray(c_a.astype(NP_BF16)))

    sched = {
        "total_slots": total_slots,
        "total_cols": total_cols,
        "chunks": chunks,
        "last_q_of_w": last_q_of_w,
    }
    return sched, idx_arrs, att_arrs, c_arrs


def _trace(sched):
    total_slots = sched["total_slots"]
    total_cols = sched["total_cols"]
    chunks = sched["chunks"]
    last_q_of_w = sched["last_q_of_w"]
    maxgt = int(os.environ.get("GNN_MAXGT", "8"))

    gq = int(os.environ.get("GNN_GQUEUES", "4"))
    scratch = int(os.environ.get("GNN_SCRATCH", "32768"))
    nc = bacc.Bacc("TRN2", debug=False, num_swdge_queues=gq,
                   dynamic_dma_scratch_size=scratch)
    embed_d = nc.dram_tensor("embed_d", [N_NODES, D], BF16,
                             kind="ExternalInput")
    x_d = nc.dram_tensor("x_d", [D, NPC_PAD], F32, kind="ExternalInput")
    idx_d = nc.dram_tensor("idx_d", [128, total_slots // 16], I16,
                           kind="ExternalInput")
    att_d = nc.dram_tensor("att_d", [128, total_cols], F32,
                           kind="ExternalInput")
    c_d = nc.dram_tensor("c_d", [128, total_cols], BF16,
                         kind="ExternalInput")
    w1t_d = nc.dram_tensor("w1t_d", [D, D], F32, kind="ExternalInput")
    w2t_d = nc.dram_tensor("w2t_d", [D, D], F32, kind="ExternalInput")
    b1_d = nc.dram_tensor("b1_d", [D, 1], F32, kind="ExternalInput")
    b2_d = nc.dram_tensor("b2_d", [D, 1], F32, kind="ExternalInput")
    out_d = nc.dram_tensor("out_d", [D, NPC], F32, kind="ExternalOutput")

    iota_np = np.tile(np.arange(W, dtype=np.float32), (128, 1)).astype(NP_BF16)

    with TileContext(nc) as tc:
        with (
            tc.tile_pool(name="aux", bufs=1) as aux,
            tc.tile_pool(name="xgp", bufs=int(os.environ.get("GNN_XGBUFS", "24"))) as xgp,
            tc.tile_pool(name="sp", bufs=int(os.environ.get("GNN_SPBUFS", "12"))) as sp,
            tc.tile_pool(name="hp", bufs=3) as hp,
            tc.tile_pool(name="ps", bufs=1, space="PSUM") as ps,
        ):
            idx_t = aux.tile([128, total_slots // 16], I16)
            nc.sync.dma_start(idx_t[:], idx_d[:])
            att_t = aux.tile([128, total_cols], F32)
            nc.sync.dma_start(att_t[:], att_d[:])
            c_t = aux.tile([128, total_cols], BF16)
            nc.sync.dma_start(c_t[:], c_d[:])
            iota_t = aux.tile([128, W], BF16)
            nc.sync.dma_start(iota_t[:], nc.inline_tensor(iota_np, "iota_i")[:])
            w1t_t = aux.tile([D, D], F32)
            nc.sync.dma_start(w1t_t[:], w1t_d[:])
            w2t_t = aux.tile([D, D], F32)
            nc.sync.dma_start(w2t_t[:], w2t_d[:])
            b1_t = aux.tile([D, 1], F32)
            nc.sync.dma_start(b1_t[:], b1_d[:])
            b2_t = aux.tile([D, 1], F32)
            nc.sync.dma_start(b2_t[:], b2_d[:])

            x_fm = aux.tile([128, NPC_PAD], F32)     # x feature-major

            phases = os.environ.get("GNN_PHASES", "ABC")

            def emit_region(r, bank):
                for ci, ch in enumerate(chunks):
                    if ch["region"] != r:
                        continue
                    n, ncols, cb = ch["n"], ch["ncols"], ch["col_base"]
                    xg = xgp.tile([128, maxgt, D], BF16, tag="xg")
                    base_ap = (embed_d[:] if ch["cls"] == 0
                               else embed_d[SPLIT:])
                    nc.gpsimd.dma_gather(
                        xg[:, :ncols, :], base_ap,
                        idx_t[:, ch["slot_base"] // 16:
                              ch["slot_base"] // 16 + ncols * 8],
                        ncols * 128, n, D,
                        elem_step=D,
                        queue_num=ch["queue"])
                    # chunk-wide S build: S[p, q, j] = (iota[j]==c[p,q])*att
                    s_t = sp.tile([128, maxgt, W], BF16, tag="s")
                    iota_b = iota_t[:].unsqueeze(1).to_broadcast(
                        [128, ncols, W])
                    c_b = c_t[:, cb:cb + ncols].unsqueeze(2).to_broadcast(
                        [128, ncols, W])
                    att_b = att_t[:, cb:cb + ncols].unsqueeze(2).to_broadcast(
                        [128, ncols, W])
                    nc.vector.tensor_tensor(
                        s_t[:, :ncols, :], iota_b, c_b,
                        op=mybir.AluOpType.is_equal)
                    # att multiply: split between DVE (chunk-wide broadcast
                    # mult) and the Activation engine (per-quantum ops with
                    # per-partition scale) to balance the two engines.
                    num = int(os.environ.get("GNN_SMUL_NUM", "1"))
                    den = int(os.environ.get("GNN_SMUL_DEN", "2"))
                    if ci % den < num:
                        nc.vector.tensor_tensor(
                            s_t[:, :ncols, :], s_t[:, :ncols, :], att_b,
                            op=mybir.AluOpType.mult)
                    else:
                        for qc2 in range(ncols):
                            nc.scalar.activation(
                                s_t[:, qc2, :], s_t[:, qc2, :],
                                mybir.ActivationFunctionType.Copy,
                                scale=att_t[:, cb + qc2:cb + qc2 + 1])
                    for qi, (qc, qa, qw) in enumerate(ch["quanta"]):
                        wl = qw % (REGION // W)
                        nc.tensor.matmul(
                            bank[:, wl * W:(wl + 1) * W],
                            lhsT=xg[:, qc, :],
                            rhs=s_t[:, qc, :],
                            start=False,
                            stop=(last_q_of_w[qw] == (ci, qi)),
                            skip_group_check=True)

            def emit_c_chunk(k, bank):
                g0 = k * REGION
                n = min(REGION, NPC_PAD - g0)
                sl = slice(g0, g0 + n)
                h1 = hp.tile([128, REGION], F32, tag="h1")
                nc.vector.tensor_add(h1[:, :n], x_fm[:, sl], bank[:, :n])
                h2 = hp.tile([128, REGION], F32, tag="h2")
                nc.vector.tensor_mul(h2[:, :n], x_fm[:, sl], bank[:, :n])
                pf1 = ps.tile([128, REGION], F32, tag="fin", bufs=2)
                nc.tensor.matmul(pf1[:, :n], lhsT=w1t_t[:], rhs=h1[:, :n],
                                 start=True, stop=True)
                o1 = hp.tile([128, REGION], F32, tag="o1")
                nc.scalar.activation(
                    o1[:, :n], pf1[:, :n], mybir.ActivationFunctionType.Lrelu,
                    bias=b1_t[:, 0:1], scale=1.0, alpha=LEAKY)
                pf2 = ps.tile([128, REGION], F32, tag="fin", bufs=2)
                nc.tensor.matmul(pf2[:, :n], lhsT=w2t_t[:], rhs=h2[:, :n],
                                 start=True, stop=True)
                o2 = hp.tile([128, REGION], F32, tag="o2")
                nc.scalar.activation(
                    o2[:, :n], pf2[:, :n], mybir.ActivationFunctionType.Lrelu,
                    bias=b2_t[:, 0:1], scale=1.0, alpha=LEAKY)
                osum = hp.tile([128, REGION], F32, tag="os")
                nc.vector.tensor_add(osum[:, :n], o1[:, :n], o2[:, :n])
                rows = min(REGION, NPC - g0)
                if rows > 0:
                    nc.sync.dma_start(out_d[:, g0:g0 + rows],
                                      osum[:, :rows])

            def body():
                if "A" in phases:
                    nc.sync.dma_start(x_fm[:], x_d[:])
                for r in range(NREG):
                    if "B" in phases:
                        bank = ps.tile([128, REGION], F32, tag="bank", bufs=4)
                        nc.scalar.memzero(bank[:])
                        emit_region(r, bank)
                    if "C" in phases and "B" in phases:
                        emit_c_chunk(r, bank)

            body()

    nc.compile()
    return nc


def kernel(entity_embed, att, src_idx, dst_idx, W1, b1, W2, b2):
    entity_embed = np.ascontiguousarray(entity_embed, dtype=np.float32)
    att = np.ascontiguousarray(att, dtype=np.float32)
    src_idx_np = np.ascontiguousarray(src_idx, dtype=np.int32)
    dst_idx_np = np.ascontiguousarray(dst_idx, dtype=np.int32)

    sched, idx_arrs, att_arrs, c_arrs = _build_schedule(
        src_idx_np, dst_idx_np, att)
    nc = _trace(sched)

    w1t = np.ascontiguousarray(np.asarray(W1, np.float32).T)
    w2t = np.ascontiguousarray(np.asarray(W2, np.float32).T)
    b1c = np.ascontiguousarray(np.asarray(b1, np.float32).reshape(D, 1))
    b2c = np.ascontiguousarray(np.asarray(b2, np.float32).reshape(D, 1))

    embed_bf16 = np.ascontiguousarray(entity_embed.astype(NP_BF16))
    in_maps = []
    for c in range(N_CORES):
        x_slice = np.zeros((NPC_PAD, D), np.float32)
        x_slice[:NPC] = entity_embed[c * NPC:(c + 1) * NPC]
        in_maps.append({
            "embed_d": embed_bf16,
            "x_d": np.ascontiguousarray(x_slice.T),
            "idx_d": idx_arrs[c],
            "att_d": att_arrs[c],
            "c_d": c_arrs[c],
            "w1t_d": w1t,
            "w2t_d": w2t,
            "b1_d": b1c,
            "b2_d": b2c,
        })

    trace = os.environ.get("GNN_KERNEL_TRACE", "0") == "1"
    res = run_bass_kernel_spmd(
        nc, in_maps, core_ids=list(range(N_CORES)), trace=trace)
    if trace and res.exec_time_ns is not None:
        print(f"HW exec time: {res.exec_time_ns} ns")

    return np.concatenate(
        [np.ascontiguousarray(res.results[c]["out_d"].T)
         for c in range(N_CORES)], axis=0)
